# revision 12
# baseline (speedup 1.0000x reference)
"""Grouped-expert SwiGLU FFN (MoE) kernel for 8 Trainium2 NeuronCores.

Problem: 8 experts, tokens pre-sorted into contiguous equal segments.
  sorted_x: (8192, 512) f32, w12: (8, 2048, 512) f32, w3: (8, 512, 1024) f32
  out[t] = SwiGLU(x[t] @ w12[e].T) @ w3[e].T   for t in expert e's segment

Sharding: expert parallelism. Core e gets its 1024-token segment plus
w12[e]/w3[e]; no collectives. Host pre-transposes operands so the
contraction dim lands on SBUF partitions (no on-chip transposes):

  GEMM1 (contract d_model=512, 4 chunks of 128):
    lhsT = w12[e].T chunk (i=128, j=128)  [stationary]
    rhs  = x_seg.T  chunk (i=128, t=512)  [moving]
    psum (j=128, t=512) accumulated over 4 i-chunks  -> h12^T orientation
  SwiGLU: silu(h1^T) * h2^T elementwise in (j, t) layout (ACT + DVE)
  GEMM2 (contract hidden=1024, 8 chunks of 128):
    lhsT = h^T chunk   (j=128, t=128)  [stationary]
    rhs  = w3[e].T chunk (j=128, o=512) [moving]
    psum (t=128, o=512) accumulated over 8 j-chunks -> natural (t, o) output

All operands are bf16 (same PE rate as f32r, half the DMA bytes; rel
err ~1e-3 vs the 2e-2 gate). The critical path is the Tensor engine:
it is 100%-busy once started, so the kernel (a) splits input loads
across both HWDGE queues (SP carries x+w3, ACT carries w12) with
issue order = consumption order so the first GEMM1 chain's operands
land ~5us earlier than a single-queue stream, and (b) runs a block of
dependency-free warm-up matmuls on zeroed scratch during the load
wait so the PE p-state ramp (0.65/1.2GHz -> 2.4GHz after ~3us busy)
is paid before real data arrives. Output is stored bf16 (halves the
tail store) and upcast on the host.
"""

import numpy as np

N_EXPERTS = 8
D_MODEL = 512
HIDDEN = 1024
TOKENS_PER_EXPERT = 1024
N_CORES = 8

_CACHE = {}


def _build_program(repeat=1, nwarm=10):
    import concourse.mybir as mybir
    import concourse.tile as tile
    from concourse import bacc

    f32 = mybir.dt.float32
    din = mybir.dt.bfloat16
    P = 128
    IC = D_MODEL // P            # 4 chunks of d_model
    JC = HIDDEN // P             # 8 chunk-pairs of 2*hidden / chunks of hidden
    WG = 2                       # w3 DMA groups
    TB = 512                     # token block (moving free dim)
    NT = TOKENS_PER_EXPERT // TB  # 2 token blocks
    TM = TOKENS_PER_EXPERT // P  # 8 output token chunks

    nc = bacc.Bacc(None, target_bir_lowering=False)

    # host-packed layouts (see _pack_inputs)
    xh_d = nc.dram_tensor("xh", [NT, IC, P, TB], din, kind="ExternalInput")
    w12_d = nc.dram_tensor("w12p", [JC, 2, P, IC * P], din, kind="ExternalInput")
    w3_d = nc.dram_tensor("w3g", [WG, JC // WG, P, D_MODEL], din, kind="ExternalInput")
    out_d = nc.dram_tensor("out", [TOKENS_PER_EXPERT, D_MODEL], din, kind="ExternalOutput")

    with tile.TileContext(nc) as tc:
        with (
            tc.tile_pool(name="persist", bufs=2) as persist,
            tc.tile_pool(name="work", bufs=3) as work,
            tc.tile_pool(name="ps1", bufs=3, space="PSUM") as ps1,
            tc.tile_pool(name="ps2", bufs=2, space="PSUM") as ps2,
        ):
            # PE warm-up: zeroed scratch (contents irrelevant, results
            # discarded); ramps the PE clock while the first input DMAs are
            # in flight. Memset on the otherwise-idle GpSimd engine; borrows
            # the ps_o buffers (free until GEMM2).
            warm = work.tile([P, TB], din, tag="warm", bufs=1, name="warm")
            nc.gpsimd.memset(warm[:], 0)
            for k in range(nwarm):
                pw = ps2.tile([P, D_MODEL], f32, tag="ps_o", name=f"warm{k}")
                nc.tensor.matmul(pw[:], warm[:, 0:P], warm[:],
                                 start=True, stop=True)

            for it in range(repeat):
                # ---- loads ----
                # One global stream in consumption order, alternated across
                # the two HWDGE queues (SP / ACT) so both sequencers issue in
                # parallel and descriptors from the head of the stream share
                # all 16 DMA engines. First bites are small (ic-pair halves
                # of x, a/b halves of w12[0]) so GEMM1 starts ~2us sooner.
                xh = [persist.tile([P, IC, TB], din, tag=f"xh{tb}",
                                   name=f"i{it}_xh{tb}") for tb in range(NT)]
                w12 = [persist.tile([P, 2, IC * P], din, tag=f"w12_{p}",
                                    name=f"i{it}_w12_{p}") for p in range(JC)]
                w3 = [persist.tile([P, JC // WG, D_MODEL], din, tag=f"w3_{g}",
                                   name=f"i{it}_w3_{g}") for g in range(WG)]
                # The SP queue starts transferring ~0.8us before the ACT
                # queue (the ACT engine runs the act-table load first), so
                # the first-consumed bites all go on SP; ACT carries the
                # later-needed bulk.
                sp_stream = [
                    (xh[0][:, 0:2, :], xh_d[0, 0:2].rearrange("i p t -> p i t")),
                    (w12[0][:, 0, :], w12_d[0, 0]),
                    (xh[0][:, 2:4, :], xh_d[0, 2:4].rearrange("i p t -> p i t")),
                    (w12[0][:, 1, :], w12_d[0, 1]),
                    (w12[1][:], w12_d[1].rearrange("h p m -> p h m")),
                    (w12[3][:], w12_d[3].rearrange("h p m -> p h m")),
                    (w12[5][:], w12_d[5].rearrange("h p m -> p h m")),
                    (w3[1][:], w3_d[1].rearrange("q p m -> p q m")),
                ]
                act_stream = [
                    (xh[1][:, 0:2, :], xh_d[1, 0:2].rearrange("i p t -> p i t")),
                    (xh[1][:, 2:4, :], xh_d[1, 2:4].rearrange("i p t -> p i t")),
                    (w12[2][:], w12_d[2].rearrange("h p m -> p h m")),
                    (w12[4][:], w12_d[4].rearrange("h p m -> p h m")),
                    (w12[6][:], w12_d[6].rearrange("h p m -> p h m")),
                    (w12[7][:], w12_d[7].rearrange("h p m -> p h m")),
                    (w3[0][:], w3_d[0].rearrange("q p m -> p q m")),
                ]
                for dst, src in sp_stream:
                    nc.sync.dma_start(dst, src)
                for dst, src in act_stream:
                    nc.scalar.dma_start(dst, src)
                h = [persist.tile([P, TOKENS_PER_EXPERT], din, tag=f"h{p}", bufs=1,
                                  name=f"i{it}_h{p}") for p in range(JC)]

                # ---- GEMM1 + SwiGLU ----
                for p in range(JC):
                    for tb in range(NT):
                        tsl = slice(tb * TB, (tb + 1) * TB)
                        ps_a = ps1.tile([P, TB], f32, tag="ps_a", name=f"i{it}_a{p}_{tb}")
                        ps_b = ps1.tile([P, TB], f32, tag="ps_b", name=f"i{it}_b{p}_{tb}")
                        for ic in range(IC):
                            nc.tensor.matmul(
                                ps_a[:], w12[p][:, 0, ic * P:(ic + 1) * P],
                                xh[tb][:, ic, :], start=(ic == 0), stop=(ic == IC - 1))
                        for ic in range(IC):
                            nc.tensor.matmul(
                                ps_b[:], w12[p][:, 1, ic * P:(ic + 1) * P],
                                xh[tb][:, ic, :], start=(ic == 0), stop=(ic == IC - 1))
                        s = work.tile([P, TB], f32, tag="silu", name=f"i{it}_s{p}_{tb}")
                        nc.scalar.activation(s[:], ps_a[:],
                                             mybir.ActivationFunctionType.Silu)
                        nc.vector.tensor_tensor(h[p][:, tsl], s[:], ps_b[:],
                                                mybir.AluOpType.mult)

                # ---- GEMM2 ----
                # Last token chunk is split into two column halves so the
                # copy+store of the first half overlaps the second half's
                # matmul chain, shortening the kernel tail.
                for tm in range(TM):
                    ps_o = ps2.tile([P, D_MODEL], f32, tag="ps_o", name=f"i{it}_o{tm}")
                    rsl = slice(tm * P, (tm + 1) * P)
                    if tm < TM - 1:
                        for jc in range(JC):
                            nc.tensor.matmul(
                                ps_o[:], h[jc][:, rsl],
                                w3[jc // (JC // WG)][:, jc % (JC // WG), :],
                                start=(jc == 0), stop=(jc == JC - 1))
                        o = work.tile([P, D_MODEL], din, tag="o", name=f"i{it}_oo{tm}")
                        nc.vector.tensor_copy(o[:], ps_o[:])
                        nc.scalar.dma_start(out_d[rsl, :], o[:])
                    else:
                        # Column halves in SEPARATE psum tiles (banks) so the
                        # first half's copy+store overlaps the second half's
                        # matmul chain (sub-tile psum deps are whole-tile).
                        HB = D_MODEL // 2
                        ps_o2 = ps2.tile([P, D_MODEL], f32, tag="ps_o",
                                         name=f"i{it}_o{tm}b")
                        for hb, pso in ((0, ps_o), (1, ps_o2)):
                            csl = slice(hb * HB, (hb + 1) * HB)
                            for jc in range(JC):
                                nc.tensor.matmul(
                                    pso[:, csl], h[jc][:, rsl],
                                    w3[jc // (JC // WG)][:, jc % (JC // WG), csl],
                                    start=(jc == 0), stop=(jc == JC - 1))
                            o = work.tile([P, HB], din, tag=f"o7_{hb}",
                                          name=f"i{it}_oo{tm}_{hb}")
                            nc.vector.tensor_copy(o[:], pso[:, csl])
                            nc.scalar.dma_start(out_d[rsl, csl], o[:])

    nc.compile()
    return nc


def _pack_inputs(sorted_x, w12, w3, starts, per):
    """Host-side shard + transpose packing for each core (bf16)."""
    import ml_dtypes
    np_dt = ml_dtypes.bfloat16
    in_maps = []
    for e in range(N_EXPERTS):
        xs = sorted_x[starts[e]:starts[e] + per]                 # (1024, 512)
        # xh[tb, ic, p, u] = x_seg[tb*512+u, ic*128+p]
        xh = np.ascontiguousarray(
            xs.T.reshape(4, 128, 2, 512).transpose(2, 0, 1, 3))
        # w12c[jc][p, ic*128+j] = w12[e][jc*128+j, ic*128+p]
        w12c = np.ascontiguousarray(
            w12[e].reshape(16, 128, 4, 128).transpose(0, 3, 2, 1)
        ).reshape(16, 128, 512)
        # pair-major: w12p[p] = stack(w12c[p], w12c[p+8]) -> (8, 2, 128, 512)
        w12p = np.ascontiguousarray(np.stack([w12c[:8], w12c[8:]], axis=1))
        # w3c[jc][k, n] = w3[e][n, jc*128+k]; grouped 4 chunks per DMA
        w3g = np.ascontiguousarray(
            w3[e].reshape(512, 8, 128).transpose(1, 2, 0)
        ).reshape(2, 4, 128, 512)
        in_maps.append({
            "xh": xh.astype(np_dt),
            "w12p": w12p.astype(np_dt),
            "w3g": w3g.astype(np_dt),
        })
    return in_maps


def _reference_numpy(sorted_x, w12, w3, expert_starts, expert_ends):
    """Exact fallback for non-canonical segment layouts."""
    x = sorted_x.astype(np.float32)
    T = x.shape[0]
    out = np.zeros((T, w3.shape[1]), dtype=np.float32)
    tok = np.arange(T)
    for e in range(w12.shape[0]):
        m = (tok >= expert_starts[e]) & (tok < expert_ends[e])
        if not m.any():
            continue
        h12 = x[m] @ w12[e].T
        h1, h2 = h12[:, :HIDDEN], h12[:, HIDDEN:]
        hact = (h1 / (1.0 + np.exp(-h1))) * h2
        out[m] += hact @ w3[e].T
    return out


def kernel(sorted_x, w12, w3, expert_starts, expert_ends):
    sorted_x = np.asarray(sorted_x)
    w12 = np.asarray(w12)
    w3 = np.asarray(w3)
    starts = np.asarray(expert_starts).astype(np.int64)
    ends = np.asarray(expert_ends).astype(np.int64)

    T = sorted_x.shape[0]
    E = w12.shape[0]
    per = T // E
    canonical = (
        E == N_EXPERTS
        and T == N_EXPERTS * TOKENS_PER_EXPERT
        and sorted_x.shape[1] == D_MODEL
        and w12.shape[1:] == (2 * HIDDEN, D_MODEL)
        and w3.shape[1:] == (D_MODEL, HIDDEN)
        and bool(np.all(starts == np.arange(E, dtype=np.int64) * per))
        and bool(np.all(ends == starts + per))
    )
    if not canonical:
        return _reference_numpy(sorted_x, w12, w3, starts, ends)

    from concourse.bass_utils import run_bass_kernel_spmd

    if "nc" not in _CACHE:
        _CACHE["nc"] = _build_program()
    nc = _CACHE["nc"]

    in_maps = _pack_inputs(sorted_x, w12, w3, starts, per)
    res = run_bass_kernel_spmd(nc, in_maps, list(range(N_CORES)))
    out = np.empty((T, D_MODEL), dtype=np.float32)
    for e in range(N_EXPERTS):
        out[e * per:(e + 1) * per] = res.results[e]["out"].astype(np.float32)
    return out


# revision 16
# speedup vs baseline: 1.0307x; 1.0307x over previous
"""Grouped-expert SwiGLU FFN (MoE) kernel for 8 Trainium2 NeuronCores.

Problem: 8 experts, tokens pre-sorted into contiguous equal segments.
  sorted_x: (8192, 512) f32, w12: (8, 2048, 512) f32, w3: (8, 512, 1024) f32
  out[t] = SwiGLU(x[t] @ w12[e].T) @ w3[e].T   for t in expert e's segment

Sharding: expert parallelism. Core e gets its 1024-token segment plus
w12[e]/w3[e]; no collectives. Host pre-transposes operands so the
contraction dim lands on SBUF partitions (no on-chip transposes):

  GEMM1 (contract d_model=512, 4 chunks of 128):
    lhsT = w12[e].T chunk (i=128, j=128)  [stationary]
    rhs  = x_seg.T  chunk (i=128, t=512)  [moving]
    psum (j=128, t=512) accumulated over 4 i-chunks  -> h12^T orientation
  SwiGLU: silu(h1^T) * h2^T elementwise in (j, t) layout (ACT + DVE)
  GEMM2 (contract hidden=1024, 8 chunks of 128):
    lhsT = h^T chunk   (j=128, t=128)  [stationary]
    rhs  = w3[e].T chunk (j=128, o=512) [moving]
    psum (t=128, o=512) accumulated over 8 j-chunks -> natural (t, o) output

All operands are bf16 (same PE rate as f32r, half the DMA bytes; rel
err ~1e-3 vs the 2e-2 gate). The critical path is the Tensor engine:
it is 100%-busy once started, so the kernel (a) splits input loads
across both HWDGE queues (SP carries x+w3, ACT carries w12) with
issue order = consumption order so the first GEMM1 chain's operands
land ~5us earlier than a single-queue stream, and (b) runs a block of
dependency-free warm-up matmuls on zeroed scratch during the load
wait so the PE p-state ramp (0.65/1.2GHz -> 2.4GHz after ~3us busy)
is paid before real data arrives. Output is stored bf16 (halves the
tail store) and upcast on the host.
"""

import numpy as np

N_EXPERTS = 8
D_MODEL = 512
HIDDEN = 1024
TOKENS_PER_EXPERT = 1024
N_CORES = 8

_CACHE = {}


def _build_program(repeat=1, nwarm=9):
    import concourse.mybir as mybir
    import concourse.tile as tile
    from concourse import bacc

    f32 = mybir.dt.float32
    din = mybir.dt.bfloat16
    P = 128
    IC = D_MODEL // P            # 4 chunks of d_model
    JC = HIDDEN // P             # 8 chunk-pairs of 2*hidden / chunks of hidden
    WG = 2                       # w3 DMA groups
    TB = 512                     # token block (moving free dim)
    NT = TOKENS_PER_EXPERT // TB  # 2 token blocks
    TM = TOKENS_PER_EXPERT // P  # 8 output token chunks

    nc = bacc.Bacc(None, target_bir_lowering=False)

    # host-packed layouts (see _pack_inputs)
    xh_d = nc.dram_tensor("xh", [NT, IC, P, TB], din, kind="ExternalInput")
    w12_d = nc.dram_tensor("w12p", [JC, 2, P, IC * P], din, kind="ExternalInput")
    w3_d = nc.dram_tensor("w3g", [WG, JC // WG, P, D_MODEL], din, kind="ExternalInput")
    out_d = nc.dram_tensor("out", [TOKENS_PER_EXPERT, D_MODEL], din, kind="ExternalOutput")

    with tile.TileContext(nc) as tc:
        with (
            tc.tile_pool(name="persist", bufs=2) as persist,
            tc.tile_pool(name="work", bufs=3) as work,
            tc.tile_pool(name="ps1", bufs=3, space="PSUM") as ps1,
            tc.tile_pool(name="ps2", bufs=2, space="PSUM") as ps2,
        ):
            # PE warm-up: scratch matmuls with no DMA deps ramp the PE
            # p-state (0.65/1.2GHz -> 2.4GHz) while the first input DMAs
            # are in flight. Iota data (not zeros) so the array sees real
            # switching activity in case the clock governor is
            # power-proportional. Borrows the ps_o buffers (free until
            # GEMM2).
            warm = work.tile([P, TB], din, tag="warm", bufs=1, name="warm")
            nc.gpsimd.iota(warm[:], [[1, TB]], base=0, channel_multiplier=7,
                           allow_small_or_imprecise_dtypes=True)
            for k in range(nwarm):
                pw = ps2.tile([P, D_MODEL], f32, tag="ps_o", name=f"warm{k}")
                nc.tensor.matmul(pw[:], warm[:, 0:P], warm[:],
                                 start=True, stop=True)

            for it in range(repeat):
                # ---- loads ----
                # One global stream in consumption order, alternated across
                # the two HWDGE queues (SP / ACT) so both sequencers issue in
                # parallel and descriptors from the head of the stream share
                # all 16 DMA engines. First bites are small (ic-pair halves
                # of x, a/b halves of w12[0]) so GEMM1 starts ~2us sooner.
                xh = [persist.tile([P, IC, TB], din, tag=f"xh{tb}",
                                   name=f"i{it}_xh{tb}") for tb in range(NT)]
                w12 = [persist.tile([P, 2, IC * P], din, tag=f"w12_{p}",
                                    name=f"i{it}_w12_{p}") for p in range(JC)]
                w3 = [persist.tile([P, JC // WG, D_MODEL], din, tag=f"w3_{g}",
                                   name=f"i{it}_w3_{g}") for g in range(WG)]
                # All loads on the SP queue, one FIFO in exact consumption
                # order (tb-outer GEMM1 keeps the early demand rate under
                # the single-queue bandwidth, with ~1us margin per item;
                # splitting across both HWDGE queues shares bandwidth
                # unpredictably and starves the head of one queue). Stores
                # go on the ACT queue.
                sp_stream = [
                    (xh[0][:, 0:2, :], xh_d[0, 0:2].rearrange("i p t -> p i t")),
                    (w12[0][:, 0, :], w12_d[0, 0]),
                    (xh[0][:, 2:4, :], xh_d[0, 2:4].rearrange("i p t -> p i t")),
                    (w12[0][:, 1, :], w12_d[0, 1]),
                ] + [
                    (w12[p][:], w12_d[p].rearrange("h p m -> p h m"))
                    for p in range(1, JC)
                ] + [
                    (xh[1][:, 0:2, :], xh_d[1, 0:2].rearrange("i p t -> p i t")),
                    (xh[1][:, 2:4, :], xh_d[1, 2:4].rearrange("i p t -> p i t")),
                ] + [
                    (w3[g][:], w3_d[g].rearrange("q p m -> p q m"))
                    for g in range(WG)
                ]
                for dst, src in sp_stream:
                    nc.sync.dma_start(dst, src)
                h = [persist.tile([P, TOKENS_PER_EXPERT], din, tag=f"h{p}", bufs=1,
                                  name=f"i{it}_h{p}") for p in range(JC)]

                # ---- GEMM1 + SwiGLU ----
                # tb-outer: the tb=0 pass touches each w12 chunk once at a
                # demand rate one DMA queue can sustain; by the tb=1 pass
                # all weights are resident.
                for tb in range(NT):
                    for p in range(JC):
                        tsl = slice(tb * TB, (tb + 1) * TB)
                        ps_a = ps1.tile([P, TB], f32, tag="ps_a", name=f"i{it}_a{p}_{tb}")
                        ps_b = ps1.tile([P, TB], f32, tag="ps_b", name=f"i{it}_b{p}_{tb}")
                        for ic in range(IC):
                            nc.tensor.matmul(
                                ps_a[:], w12[p][:, 0, ic * P:(ic + 1) * P],
                                xh[tb][:, ic, :], start=(ic == 0), stop=(ic == IC - 1))
                        for ic in range(IC):
                            nc.tensor.matmul(
                                ps_b[:], w12[p][:, 1, ic * P:(ic + 1) * P],
                                xh[tb][:, ic, :], start=(ic == 0), stop=(ic == IC - 1))
                        s = work.tile([P, TB], f32, tag="silu", name=f"i{it}_s{p}_{tb}")
                        nc.scalar.activation(s[:], ps_a[:],
                                             mybir.ActivationFunctionType.Silu)
                        nc.vector.tensor_tensor(h[p][:, tsl], s[:], ps_b[:],
                                                mybir.AluOpType.mult)

                # ---- GEMM2 ----
                # Last token chunk is split into two column halves so the
                # copy+store of the first half overlaps the second half's
                # matmul chain, shortening the kernel tail.
                for tm in range(TM):
                    ps_o = ps2.tile([P, D_MODEL], f32, tag="ps_o", name=f"i{it}_o{tm}")
                    rsl = slice(tm * P, (tm + 1) * P)
                    if tm < TM - 1:
                        for jc in range(JC):
                            nc.tensor.matmul(
                                ps_o[:], h[jc][:, rsl],
                                w3[jc // (JC // WG)][:, jc % (JC // WG), :],
                                start=(jc == 0), stop=(jc == JC - 1))
                        o = work.tile([P, D_MODEL], din, tag="o", name=f"i{it}_oo{tm}")
                        nc.vector.tensor_copy(o[:], ps_o[:])
                        nc.scalar.dma_start(out_d[rsl, :], o[:])
                    else:
                        # Column halves in SEPARATE psum tiles (banks) so the
                        # first half's copy+store overlaps the second half's
                        # matmul chain (sub-tile psum deps are whole-tile).
                        HB = D_MODEL // 2
                        ps_o2 = ps2.tile([P, D_MODEL], f32, tag="ps_o",
                                         name=f"i{it}_o{tm}b")
                        for hb, pso in ((0, ps_o), (1, ps_o2)):
                            csl = slice(hb * HB, (hb + 1) * HB)
                            for jc in range(JC):
                                nc.tensor.matmul(
                                    pso[:, csl], h[jc][:, rsl],
                                    w3[jc // (JC // WG)][:, jc % (JC // WG), csl],
                                    start=(jc == 0), stop=(jc == JC - 1))
                            o = work.tile([P, HB], din, tag=f"o7_{hb}",
                                          name=f"i{it}_oo{tm}_{hb}")
                            nc.vector.tensor_copy(o[:], pso[:, csl])
                            nc.scalar.dma_start(out_d[rsl, csl], o[:])

    nc.compile()
    return nc


def _pack_inputs(sorted_x, w12, w3, starts, per):
    """Host-side shard + transpose packing for each core (bf16)."""
    import ml_dtypes
    np_dt = ml_dtypes.bfloat16
    in_maps = []
    for e in range(N_EXPERTS):
        xs = sorted_x[starts[e]:starts[e] + per]                 # (1024, 512)
        # xh[tb, ic, p, u] = x_seg[tb*512+u, ic*128+p]
        xh = np.ascontiguousarray(
            xs.T.reshape(4, 128, 2, 512).transpose(2, 0, 1, 3))
        # w12c[jc][p, ic*128+j] = w12[e][jc*128+j, ic*128+p]
        w12c = np.ascontiguousarray(
            w12[e].reshape(16, 128, 4, 128).transpose(0, 3, 2, 1)
        ).reshape(16, 128, 512)
        # pair-major: w12p[p] = stack(w12c[p], w12c[p+8]) -> (8, 2, 128, 512)
        w12p = np.ascontiguousarray(np.stack([w12c[:8], w12c[8:]], axis=1))
        # w3c[jc][k, n] = w3[e][n, jc*128+k]; grouped 4 chunks per DMA
        w3g = np.ascontiguousarray(
            w3[e].reshape(512, 8, 128).transpose(1, 2, 0)
        ).reshape(2, 4, 128, 512)
        in_maps.append({
            "xh": xh.astype(np_dt),
            "w12p": w12p.astype(np_dt),
            "w3g": w3g.astype(np_dt),
        })
    return in_maps


def _reference_numpy(sorted_x, w12, w3, expert_starts, expert_ends):
    """Exact fallback for non-canonical segment layouts."""
    x = sorted_x.astype(np.float32)
    T = x.shape[0]
    out = np.zeros((T, w3.shape[1]), dtype=np.float32)
    tok = np.arange(T)
    for e in range(w12.shape[0]):
        m = (tok >= expert_starts[e]) & (tok < expert_ends[e])
        if not m.any():
            continue
        h12 = x[m] @ w12[e].T
        h1, h2 = h12[:, :HIDDEN], h12[:, HIDDEN:]
        hact = (h1 / (1.0 + np.exp(-h1))) * h2
        out[m] += hact @ w3[e].T
    return out


def kernel(sorted_x, w12, w3, expert_starts, expert_ends):
    sorted_x = np.asarray(sorted_x)
    w12 = np.asarray(w12)
    w3 = np.asarray(w3)
    starts = np.asarray(expert_starts).astype(np.int64)
    ends = np.asarray(expert_ends).astype(np.int64)

    T = sorted_x.shape[0]
    E = w12.shape[0]
    per = T // E
    canonical = (
        E == N_EXPERTS
        and T == N_EXPERTS * TOKENS_PER_EXPERT
        and sorted_x.shape[1] == D_MODEL
        and w12.shape[1:] == (2 * HIDDEN, D_MODEL)
        and w3.shape[1:] == (D_MODEL, HIDDEN)
        and bool(np.all(starts == np.arange(E, dtype=np.int64) * per))
        and bool(np.all(ends == starts + per))
    )
    if not canonical:
        return _reference_numpy(sorted_x, w12, w3, starts, ends)

    from concourse.bass_utils import run_bass_kernel_spmd

    if "nc" not in _CACHE:
        _CACHE["nc"] = _build_program()
    nc = _CACHE["nc"]

    in_maps = _pack_inputs(sorted_x, w12, w3, starts, per)
    res = run_bass_kernel_spmd(nc, in_maps, list(range(N_CORES)))
    out = np.empty((T, D_MODEL), dtype=np.float32)
    for e in range(N_EXPERTS):
        out[e * per:(e + 1) * per] = res.results[e]["out"].astype(np.float32)
    return out


# revision 24
# speedup vs baseline: 1.0470x; 1.0159x over previous
"""Grouped-expert SwiGLU FFN (MoE) kernel for 8 Trainium2 NeuronCores.

Problem: 8 experts, tokens pre-sorted into contiguous equal segments.
  sorted_x: (8192, 512) f32, w12: (8, 2048, 512) f32, w3: (8, 512, 1024) f32
  out[t] = SwiGLU(x[t] @ w12[e].T) @ w3[e].T   for t in expert e's segment

Sharding: expert parallelism. Core e gets its 1024-token segment plus
w12[e]/w3[e]; no collectives. Host pre-transposes operands so the
contraction dim lands on SBUF partitions (no on-chip transposes):

  GEMM1 (contract d_model=512, 4 chunks of 128):
    lhsT = w12[e].T chunk (i=128, j=128)  [stationary]
    rhs  = x_seg.T  chunk (i=128, t=512)  [moving]
    psum (j=128, t=512) accumulated over 4 i-chunks  -> h12^T orientation
  SwiGLU: silu(h1^T) * h2^T elementwise in (j, t) layout (ACT + DVE)
  GEMM2 (contract hidden=1024, 8 chunks of 128):
    lhsT = h^T chunk   (j=128, t=128)  [stationary]
    rhs  = w3[e].T chunk (j=128, o=512) [moving]
    psum (t=128, o=512) accumulated over 8 j-chunks -> natural (t, o) output

All operands are bf16 (same PE rate as f32r, half the DMA bytes; rel
err ~1e-3 vs the 2e-2 gate). The critical path is the Tensor engine:
it is 100%-busy once started, so the kernel (a) splits input loads
across both HWDGE queues (SP carries x+w3, ACT carries w12) with
issue order = consumption order so the first GEMM1 chain's operands
land ~5us earlier than a single-queue stream, and (b) runs a block of
dependency-free warm-up matmuls on zeroed scratch during the load
wait so the PE p-state ramp (0.65/1.2GHz -> 2.4GHz after ~3us busy)
is paid before real data arrives. Output is stored bf16 (halves the
tail store) and upcast on the host.
"""

import numpy as np

N_EXPERTS = 8
D_MODEL = 512
HIDDEN = 1024
TOKENS_PER_EXPERT = 1024
N_CORES = 8

_CACHE = {}


def _build_program(repeat=1, nwarm=8):
    import concourse.mybir as mybir
    import concourse.tile as tile
    from concourse import bacc

    f32 = mybir.dt.float32
    din = mybir.dt.bfloat16
    P = 128
    IC = D_MODEL // P            # 4 chunks of d_model
    JC = HIDDEN // P             # 8 chunk-pairs of 2*hidden / chunks of hidden
    WG = 2                       # w3 DMA groups
    TB = 512                     # token block (moving free dim)
    NT = TOKENS_PER_EXPERT // TB  # 2 token blocks
    TM = TOKENS_PER_EXPERT // P  # 8 output token chunks

    nc = bacc.Bacc(None, target_bir_lowering=False)

    # host-packed layouts (see _pack_inputs). "head" packs the first
    # working set (x token-block 0 + w12[0]) as two wide-descriptor DMAs:
    # head[0] = [x_ic0 | x_ic1 | w12_0a], head[1] = [x_ic2 | x_ic3 | w12_0b]
    head_d = nc.dram_tensor("head", [2, P, 3 * TB], din, kind="ExternalInput")
    x1_d = nc.dram_tensor("x1", [IC, P, TB], din, kind="ExternalInput")
    w12_d = nc.dram_tensor("w12p", [JC, 2, P, IC * P], din, kind="ExternalInput")
    w3_d = nc.dram_tensor("w3g", [WG, JC // WG, P, D_MODEL], din, kind="ExternalInput")
    out_d = nc.dram_tensor("out", [TOKENS_PER_EXPERT, D_MODEL], din, kind="ExternalOutput")

    with tile.TileContext(nc) as tc:
        with (
            tc.tile_pool(name="persist", bufs=2) as persist,
            tc.tile_pool(name="work", bufs=3) as work,
            tc.tile_pool(name="ps1", bufs=3, space="PSUM") as ps1,
            tc.tile_pool(name="ps2", bufs=2, space="PSUM") as ps2,
        ):
            # PE warm-up: scratch matmuls with no DMA deps ramp the PE
            # p-state (0.65/1.2GHz -> 2.4GHz) while the first input DMAs
            # are in flight. Iota data (not zeros) so the array sees real
            # switching activity in case the clock governor is
            # power-proportional. Borrows the ps_o buffers (free until
            # GEMM2).
            # Two-stage: a small iota first so the first (narrow) warm-up
            # matmuls start ~1us earlier; the full-width iota lands while
            # they run.
            warm = work.tile([P, TB], din, tag="warm", bufs=1, name="warm")
            nc.gpsimd.iota(warm[:, 0:P], [[1, P]], base=0, channel_multiplier=7,
                           allow_small_or_imprecise_dtypes=True)
            nc.gpsimd.iota(warm[:, P:TB], [[1, TB - P]], base=3,
                           channel_multiplier=5,
                           allow_small_or_imprecise_dtypes=True)
            for k in range(nwarm):
                pw = ps2.tile([P, D_MODEL], f32, tag="ps_o", name=f"warm{k}")
                mov = warm[:, 0:P] if k < 3 else warm[:]
                nc.tensor.matmul(pw[:, 0:mov.shape[-1]], warm[:, 0:P], mov,
                                 start=True, stop=True)

            for it in range(repeat):
                # ---- loads ----
                # All loads on the SP queue, one FIFO in exact consumption
                # order (tb-outer GEMM1 keeps the early demand rate under
                # the single-queue bandwidth; splitting across both HWDGE
                # queues shares bandwidth unpredictably and starves the
                # head of one queue). The two "head" DMAs carry the whole
                # first working set with 3KB descriptors and per-half
                # semaphores. Stores go on the ACT queue.
                hd = [persist.tile([P, 3 * TB], din, tag=f"head{k}",
                                   name=f"i{it}_head{k}") for k in range(2)]
                x1 = persist.tile([P, IC, TB], din, tag="x1", name=f"i{it}_x1")
                w12 = [persist.tile([P, 2, IC * P], din, tag=f"w12_{p}",
                                    name=f"i{it}_w12_{p}") for p in range(1, JC)]
                w3 = [persist.tile([P, JC // WG, D_MODEL], din, tag=f"w3_{g}",
                                   name=f"i{it}_w3_{g}") for g in range(WG)]
                sp_stream = [
                    (hd[0][:], head_d[0]),
                    (hd[1][:], head_d[1]),
                ] + [
                    (w12[p - 1][:], w12_d[p].rearrange("h p m -> p h m"))
                    for p in range(1, JC)
                ] + [
                    (x1[:], x1_d.rearrange("i p t -> p i t")),
                ] + [
                    (w3[g][:], w3_d[g].rearrange("q p m -> p q m"))
                    for g in range(WG)
                ]
                for dst, src in sp_stream:
                    nc.sync.dma_start(dst, src)
                h = [persist.tile([P, TOKENS_PER_EXPERT], din, tag=f"h{p}", bufs=1,
                                  name=f"i{it}_h{p}") for p in range(JC)]

                def x_ap(tb, ic):
                    if tb == 1:
                        return x1[:, ic, :]
                    return hd[ic // 2][:, (ic % 2) * TB:(ic % 2 + 1) * TB]

                def w12_ap(p, half, ic):
                    if p == 0:
                        return hd[half][:, 2 * TB + ic * P:2 * TB + (ic + 1) * P]
                    return w12[p - 1][:, half, ic * P:(ic + 1) * P]

                # ---- GEMM1 + SwiGLU ----
                # tb-outer: the tb=0 pass touches each w12 chunk once at a
                # demand rate one DMA queue can sustain; by the tb=1 pass
                # all weights are resident.
                for tb in range(NT):
                    for p in range(JC):
                        tsl = slice(tb * TB, (tb + 1) * TB)
                        ps_a = ps1.tile([P, TB], f32, tag="ps_a", name=f"i{it}_a{p}_{tb}")
                        ps_b = ps1.tile([P, TB], f32, tag="ps_b", name=f"i{it}_b{p}_{tb}")
                        for ic in range(IC):
                            nc.tensor.matmul(
                                ps_a[:], w12_ap(p, 0, ic), x_ap(tb, ic),
                                start=(ic == 0), stop=(ic == IC - 1))
                        for ic in range(IC):
                            nc.tensor.matmul(
                                ps_b[:], w12_ap(p, 1, ic), x_ap(tb, ic),
                                start=(ic == 0), stop=(ic == IC - 1))
                        s = work.tile([P, TB], f32, tag="silu", name=f"i{it}_s{p}_{tb}")
                        nc.scalar.activation(s[:], ps_a[:],
                                             mybir.ActivationFunctionType.Silu)
                        nc.vector.tensor_tensor(h[p][:, tsl], s[:], ps_b[:],
                                                mybir.AluOpType.mult)

                # ---- GEMM2 ----
                # Last token chunk is split into two column halves so the
                # copy+store of the first half overlaps the second half's
                # matmul chain, shortening the kernel tail.
                for tm in range(TM):
                    ps_o = ps2.tile([P, D_MODEL], f32, tag="ps_o", name=f"i{it}_o{tm}")
                    rsl = slice(tm * P, (tm + 1) * P)
                    if tm < TM - 1:
                        for jc in range(JC):
                            nc.tensor.matmul(
                                ps_o[:], h[jc][:, rsl],
                                w3[jc // (JC // WG)][:, jc % (JC // WG), :],
                                start=(jc == 0), stop=(jc == JC - 1))
                        o = work.tile([P, D_MODEL], din, tag="o", name=f"i{it}_oo{tm}")
                        nc.vector.tensor_copy(o[:], ps_o[:])
                        nc.scalar.dma_start(out_d[rsl, :], o[:])
                    else:
                        # Column halves in SEPARATE psum tiles (banks) so the
                        # first half's copy+store overlaps the second half's
                        # matmul chain (sub-tile psum deps are whole-tile).
                        HB = D_MODEL // 2
                        ps_o2 = ps2.tile([P, D_MODEL], f32, tag="ps_o",
                                         name=f"i{it}_o{tm}b")
                        for hb, pso in ((0, ps_o), (1, ps_o2)):
                            csl = slice(hb * HB, (hb + 1) * HB)
                            for jc in range(JC):
                                nc.tensor.matmul(
                                    pso[:, csl], h[jc][:, rsl],
                                    w3[jc // (JC // WG)][:, jc % (JC // WG), csl],
                                    start=(jc == 0), stop=(jc == JC - 1))
                            o = work.tile([P, HB], din, tag=f"o7_{hb}",
                                          name=f"i{it}_oo{tm}_{hb}")
                            nc.vector.tensor_copy(o[:], pso[:, csl])
                            nc.scalar.dma_start(out_d[rsl, csl], o[:])

    nc.compile()
    return nc


def _pack_inputs(sorted_x, w12, w3, starts, per):
    """Host-side shard + transpose packing for each core (bf16)."""
    import ml_dtypes
    np_dt = ml_dtypes.bfloat16
    in_maps = []
    for e in range(N_EXPERTS):
        xs = sorted_x[starts[e]:starts[e] + per]                 # (1024, 512)
        # xh[tb, ic, p, u] = x_seg[tb*512+u, ic*128+p]
        xh = np.ascontiguousarray(
            xs.T.reshape(4, 128, 2, 512).transpose(2, 0, 1, 3))
        # w12c[jc][p, ic*128+j] = w12[e][jc*128+j, ic*128+p]
        w12c = np.ascontiguousarray(
            w12[e].reshape(16, 128, 4, 128).transpose(0, 3, 2, 1)
        ).reshape(16, 128, 512)
        # pair-major: w12p[p] = stack(w12c[p], w12c[p+8]) -> (8, 2, 128, 512)
        w12p = np.ascontiguousarray(np.stack([w12c[:8], w12c[8:]], axis=1))
        # head[k] = [x(tb0, ic=2k) | x(tb0, ic=2k+1) | w12_0 half k]
        head = np.stack([
            np.concatenate([xh[0, 0], xh[0, 1], w12p[0, 0]], axis=1),
            np.concatenate([xh[0, 2], xh[0, 3], w12p[0, 1]], axis=1),
        ])
        # w3c[jc][k, n] = w3[e][n, jc*128+k]; grouped 4 chunks per DMA
        w3g = np.ascontiguousarray(
            w3[e].reshape(512, 8, 128).transpose(1, 2, 0)
        ).reshape(2, 4, 128, 512)
        in_maps.append({
            "head": head.astype(np_dt),
            "x1": np.ascontiguousarray(xh[1]).astype(np_dt),
            "w12p": w12p.astype(np_dt),
            "w3g": w3g.astype(np_dt),
        })
    return in_maps


def _reference_numpy(sorted_x, w12, w3, expert_starts, expert_ends):
    """Exact fallback for non-canonical segment layouts."""
    x = sorted_x.astype(np.float32)
    T = x.shape[0]
    out = np.zeros((T, w3.shape[1]), dtype=np.float32)
    tok = np.arange(T)
    for e in range(w12.shape[0]):
        m = (tok >= expert_starts[e]) & (tok < expert_ends[e])
        if not m.any():
            continue
        h12 = x[m] @ w12[e].T
        h1, h2 = h12[:, :HIDDEN], h12[:, HIDDEN:]
        hact = (h1 / (1.0 + np.exp(-h1))) * h2
        out[m] += hact @ w3[e].T
    return out


def kernel(sorted_x, w12, w3, expert_starts, expert_ends):
    sorted_x = np.asarray(sorted_x)
    w12 = np.asarray(w12)
    w3 = np.asarray(w3)
    starts = np.asarray(expert_starts).astype(np.int64)
    ends = np.asarray(expert_ends).astype(np.int64)

    T = sorted_x.shape[0]
    E = w12.shape[0]
    per = T // E
    canonical = (
        E == N_EXPERTS
        and T == N_EXPERTS * TOKENS_PER_EXPERT
        and sorted_x.shape[1] == D_MODEL
        and w12.shape[1:] == (2 * HIDDEN, D_MODEL)
        and w3.shape[1:] == (D_MODEL, HIDDEN)
        and bool(np.all(starts == np.arange(E, dtype=np.int64) * per))
        and bool(np.all(ends == starts + per))
    )
    if not canonical:
        return _reference_numpy(sorted_x, w12, w3, starts, ends)

    from concourse.bass_utils import run_bass_kernel_spmd

    if "nc" not in _CACHE:
        _CACHE["nc"] = _build_program()
    nc = _CACHE["nc"]

    in_maps = _pack_inputs(sorted_x, w12, w3, starts, per)
    res = run_bass_kernel_spmd(nc, in_maps, list(range(N_CORES)))
    out = np.empty((T, D_MODEL), dtype=np.float32)
    for e in range(N_EXPERTS):
        out[e * per:(e + 1) * per] = res.results[e]["out"].astype(np.float32)
    return out
